# revision 5
# baseline (speedup 1.0000x reference)
"""Trainium2 Bass kernel for nn_PinnLayer (PINN power-grid layer).

Math (per batch row b, closed-form nested forward-mode AD wrt t):
  x = [tn, pn] in R^513, tn = 0.2*t - 1, pn = alpha*p + beta
  z1 = x W0 + b0;  zdot1 = 0.2*W0[0,:] =: r1 (const);  zddot1 = 0
  a  = tanh(z);  u = 1-a^2;  adot = u*zdot;  addot = u*zddot - 2*a*u*zdot^2
  (3 tanh layers), out = a3 Wout + bout, out_t = ad3 Wout, out_tt = add3 Wout
  conn_i = sum_j lb[i,j] sin(o_i - o_j) = sin(o_i)*(lb cos(o))_i - cos(o_i)*(lb sin(o))_i
  physics = lam_m*out_tt + lam_d*out_t + conn - p

Device layout: everything transposed — hidden/bus dim on partitions (4 chunks
of 128), batch on the free dim (128 per core, data-parallel over 8 cores).
Weights W[k_in, m_out] are used directly as matmul lhsT; activations never
need transposing. Per layer the rhs is the stacked [a | adot | addot]
(free=384) so each weight chunk is loaded once for all three matmuls.

The layer-1 matmul runs in exact fp32 (its inputs are the raw normalized
network inputs). Layers 2/3/out/conn optionally run in float32r (single-pass
PE streaming, ~4x the fp32 matmul rate at free>=256); all producers of those
matmul operands write float32r so the HW rounds consistently.
"""

import numpy as np

import concourse.bass as bass
import concourse.tile as tile
import concourse.mybir as mybir
from concourse import bacc
from concourse.bass import ts
from concourse.bass_utils import run_bass_kernel_spmd

F32 = mybir.dt.float32
F32R = mybir.dt.float32r
AF = mybir.ActivationFunctionType
OP = mybir.AluOpType

B, N, H = 1024, 512, 512
NCORES = 8
BT = B // NCORES          # 128 batch per core
C = 4                     # 128-partition chunks over H / N

# cst column layout: [128, 4]-shaped blocks at 4*i, then single columns
CB0, CB1, CB2, CBO, CLM, CLD, CAL, CBE, CR1, CS1 = (4 * i for i in range(10))
CZERO, CHALFPI, CONE = 40, 41, 42
NCST = 43

MM_DT = F32R


def build_nc(mm_dt=MM_DT):
    nc = bacc.Bacc("TRN2", target_bir_lowering=False, debug=False)

    tn_d = nc.dram_tensor("tn", [1, BT], F32, kind="ExternalInput").ap()
    pT_d = nc.dram_tensor("pT", [N, BT], F32, kind="ExternalInput").ap()
    w0r_d = nc.dram_tensor("w0r", [1, H], F32, kind="ExternalInput").ap()
    w0b_d = nc.dram_tensor("w0b", [N, H], F32, kind="ExternalInput").ap()
    w1_d = nc.dram_tensor("w1", [H, H], mm_dt, kind="ExternalInput").ap()
    w2_d = nc.dram_tensor("w2", [H, H], mm_dt, kind="ExternalInput").ap()
    wo_d = nc.dram_tensor("wo", [H, N], mm_dt, kind="ExternalInput").ap()
    lbT_d = nc.dram_tensor("lbT", [N, N], mm_dt, kind="ExternalInput").ap()
    cst_d = nc.dram_tensor("cst", [128, NCST], F32, kind="ExternalInput").ap()

    outT_d = nc.dram_tensor("outT", [N, BT], F32, kind="ExternalOutput").ap()
    out_tT_d = nc.dram_tensor("out_tT", [N, BT], F32, kind="ExternalOutput").ap()
    physT_d = nc.dram_tensor("physT", [N, BT], F32, kind="ExternalOutput").ap()

    # read-side view of an R-layer tile for non-matmul consumers: the bits
    # are already rounded, read them as plain fp32
    rd = (lambda ap: ap.bitcast(F32)) if mm_dt != F32 else (lambda ap: ap)

    with tile.TileContext(nc) as tc:
        with (
            tc.tile_pool(name="weights", bufs=1) as wp,
            tc.tile_pool(name="data", bufs=1) as dp,
            tc.tile_pool(name="scratch", bufs=4) as sp,
            tc.tile_pool(name="psum", bufs=4, space="PSUM") as pp,
            tc.tile_pool(name="psum2", bufs=4, space="PSUM") as pp2,
        ):
            # ---- input DMAs (one per tensor; weights land as [128, C*out]) --
            cst = dp.tile([128, NCST], F32)
            nc.sync.dma_start(cst[:], cst_d)
            tn = dp.tile([1, BT], F32)
            nc.sync.dma_start(tn[:], tn_d)
            pT = dp.tile([128, C * BT], F32)
            nc.sync.dma_start(
                pT[:].rearrange("p (c b) -> p c b", b=BT),
                pT_d.rearrange("(c p) b -> p c b", p=128),
            )
            w0r = wp.tile([1, H], F32)
            nc.sync.dma_start(w0r[:], w0r_d)

            def load_w(name, d, dt):
                t = wp.tile([128, C * H], dt, tag=name)
                nc.sync.dma_start(
                    t[:].rearrange("p (c n) -> p c n", n=H),
                    d.rearrange("(c p) n -> p c n", p=128),
                )
                return t

            w0 = load_w("w0", w0b_d, F32)
            w1 = load_w("w1", w1_d, mm_dt)
            w2 = load_w("w2", w2_d, mm_dt)
            wo = load_w("wo", wo_d, mm_dt)
            lb = load_w("lb", lbT_d, mm_dt)

            def col(base, m=0):
                return cst[:, base + m : base + m + 1]

            zero = col(CZERO)
            halfpi = col(CHALFPI)
            one = col(CONE)

            # ---- normalize power: pn = alpha*p + beta ----------------------
            pn = dp.tile([128, C * BT], F32)
            for k in range(C):
                nc.vector.tensor_scalar(
                    pn[:, ts(k, BT)], pT[:, ts(k, BT)],
                    col(CAL, k), col(CBE, k), OP.mult, OP.add,
                )

            # ---- layer 1 (exact fp32 matmul) -------------------------------
            # R tiles hold [a | adot | addot] per chunk, free-stacked (384)
            R1 = dp.tile([128, C * 384], mm_dt, tag="R1")
            for m in range(C):
                ps = pp.tile([128, 384], F32, tag="ps")
                z = ps[:, 0:128]
                for k in range(C):
                    nc.tensor.matmul(
                        z, w0[:, k * H + m * 128 : k * H + (m + 1) * 128],
                        pn[:, ts(k, BT)], start=(k == 0), stop=False,
                    )
                nc.tensor.matmul(
                    z, w0r[0:1, ts(m, 128)], tn[0:1, :], start=False, stop=True,
                )
                A = R1[:, m * 384 : m * 384 + 128]
                D = R1[:, m * 384 + 128 : m * 384 + 256]
                DD = R1[:, m * 384 + 256 : m * 384 + 384]
                nc.scalar.activation(A, z, AF.Tanh, bias=col(CB0, m))
                sq = sp.tile([128, 128], F32, tag="sq")
                nc.scalar.activation(sq[:], rd(A), AF.Square, bias=zero)
                u = sp.tile([128, 128], F32, tag="u")
                nc.scalar.activation(u[:], sq[:], AF.Identity, bias=one, scale=-1.0)
                # adot = u * r1 ; addot = (adot * s1) * a  with s1 = -2*r1
                nc.gpsimd.tensor_scalar_mul(D, u[:], col(CR1, m))
                d1 = sp.tile([128, 128], F32, tag="d1")
                nc.gpsimd.tensor_scalar_mul(d1[:], rd(D), col(CS1, m))
                nc.gpsimd.tensor_mul(DD, d1[:], rd(A))

            # ---- layers 2, 3 ----------------------------------------------
            Rp = R1
            for w, cb, rtag in ((w1, CB1, "R2"), (w2, CB2, "R3")):
                Rn = dp.tile([128, C * 384], mm_dt, tag=rtag)
                for m in range(C):
                    ps = pp.tile([128, 384], F32, tag="ps")
                    for k in range(C):
                        nc.tensor.matmul(
                            ps[:],
                            w[:, k * H + m * 128 : k * H + (m + 1) * 128],
                            Rp[:, k * 384 : (k + 1) * 384],
                            start=(k == 0), stop=(k == C - 1),
                        )
                    z, zd, zdd = ps[:, 0:128], ps[:, 128:256], ps[:, 256:384]
                    A = Rn[:, m * 384 : m * 384 + 128]
                    D = Rn[:, m * 384 + 128 : m * 384 + 256]
                    DD = Rn[:, m * 384 + 256 : m * 384 + 384]
                    nc.scalar.activation(A, z, AF.Tanh, bias=col(cb, m))
                    sq = sp.tile([128, 128], F32, tag="sq")
                    nc.scalar.activation(sq[:], rd(A), AF.Square, bias=zero)
                    u = sp.tile([128, 128], F32, tag="u")
                    nc.scalar.activation(u[:], sq[:], AF.Identity, bias=one, scale=-1.0)
                    nc.vector.tensor_mul(D, u[:], zd)
                    q = sp.tile([128, 128], F32, tag="q")
                    nc.vector.tensor_mul(q[:], rd(D), zd)         # u*zd^2
                    t2 = sp.tile([128, 128], F32, tag="t2")
                    nc.gpsimd.tensor_scalar_mul(t2[:], q[:], -2.0)
                    nc.gpsimd.tensor_mul(t2[:], t2[:], rd(A))     # -2*a*u*zd^2
                    nc.vector.tensor_mul(DD, u[:], zdd)
                    nc.gpsimd.tensor_add(DD, rd(DD), t2[:])
                Rp = Rn

            # ---- output layer ----------------------------------------------
            O = dp.tile([128, C * BT], F32, tag="O")
            OT = dp.tile([128, C * BT], F32, tag="OT")
            PH = dp.tile([128, C * BT], F32, tag="PH")
            SC = dp.tile([128, C * 256], mm_dt, tag="SC")
            for m in range(C):
                ps = pp.tile([128, 384], F32, tag="ps")
                for k in range(C):
                    nc.tensor.matmul(
                        ps[:],
                        wo[:, k * H + m * 128 : k * H + (m + 1) * 128],
                        Rp[:, k * 384 : (k + 1) * 384],
                        start=(k == 0), stop=(k == C - 1),
                    )
                o = O[:, ts(m, BT)]
                ot = OT[:, ts(m, BT)]
                ph = PH[:, ts(m, BT)]
                nc.vector.tensor_scalar_add(o, ps[:, 0:128], col(CBO, m))
                nc.scalar.copy(ot, ps[:, 128:256])
                # ph = lam_m*out_tt - p, then += lam_d*out_t
                nc.vector.scalar_tensor_tensor(
                    ph, ps[:, 256:384], col(CLM, m), pT[:, ts(m, BT)],
                    OP.mult, OP.subtract)
                nc.vector.scalar_tensor_tensor(
                    ph, ps[:, 128:256], col(CLD, m), ph, OP.mult, OP.add)
                S = SC[:, m * 256 : m * 256 + 128]
                Cc = SC[:, m * 256 + 128 : m * 256 + 256]
                nc.scalar.activation(S, o, AF.Sin, bias=zero)
                nc.scalar.activation(Cc, o, AF.Sin, bias=halfpi)

            # ---- connectivity: conn = S*(lb C) - C*(lb S) -------------------
            for m in range(C):
                ps2 = pp2.tile([128, 256], F32, tag="ps2")
                for k in range(C):
                    nc.tensor.matmul(
                        ps2[:],
                        lb[:, k * H + m * 128 : k * H + (m + 1) * 128],
                        SC[:, k * 256 : (k + 1) * 256],
                        start=(k == 0), stop=(k == C - 1),
                    )
                SMt, CMt = ps2[:, 0:128], ps2[:, 128:256]
                S = rd(SC[:, m * 256 : m * 256 + 128])
                Cc = rd(SC[:, m * 256 + 128 : m * 256 + 256])
                ph = PH[:, ts(m, BT)]
                q2 = sp.tile([128, 128], F32, tag="q2")
                nc.vector.tensor_mul(q2[:], S, CMt)
                nc.gpsimd.tensor_add(ph, ph, q2[:])
                q3 = sp.tile([128, 128], F32, tag="q3")
                nc.vector.tensor_mul(q3[:], Cc, SMt)
                nc.gpsimd.tensor_sub(ph, ph, q3[:])

            # ---- output DMAs -----------------------------------------------
            for d, t in ((outT_d, O), (out_tT_d, OT), (physT_d, PH)):
                nc.sync.dma_start(
                    d.rearrange("(c p) b -> p c b", p=128),
                    t[:].rearrange("p (c b) -> p c b", b=BT),
                )

    nc.compile()
    return nc


def _host_prep(inputs):
    f = lambda x: np.ascontiguousarray(np.asarray(x, dtype=np.float32))
    t = f(inputs["time_input"])          # [B,1]
    p = f(inputs["power_input"])         # [B,N]
    W0 = f(inputs["W0"])
    pl = f(inputs["p_lower"]).reshape(-1)
    pu = f(inputs["p_upper"]).reshape(-1)

    no_var = pu == pl
    denom = np.where(no_var, 1.0, pu - pl).astype(np.float32)
    alpha = np.where(no_var, 0.0, 2.0 / denom).astype(np.float32)
    beta = np.where(no_var, 0.0, -2.0 * pl / denom - 1.0).astype(np.float32)

    r1 = (0.2 * W0[0, :]).astype(np.float32)
    s1 = (-2.0 * r1).astype(np.float32)

    def colpack(v):
        return np.asarray(v, np.float32).reshape(C, 128).T  # [128, 4]

    blocks = [
        colpack(inputs["b0"]), colpack(inputs["b1"]), colpack(inputs["b2"]),
        colpack(inputs["bout"]), colpack(np.asarray(inputs["lambda_m"]).reshape(-1)),
        colpack(np.asarray(inputs["lambda_d"]).reshape(-1)),
        colpack(alpha), colpack(beta), colpack(r1), colpack(s1),
        np.zeros((128, 1), np.float32),
        np.full((128, 1), np.pi / 2, np.float32),
        np.ones((128, 1), np.float32),
    ]
    cst = np.ascontiguousarray(np.concatenate(blocks, axis=1).astype(np.float32))

    tnT = np.ascontiguousarray((0.2 * t - 1.0).T)        # [1, B]
    pT = np.ascontiguousarray(p.T)                        # [N, B]

    shared = {
        "w0r": np.ascontiguousarray(W0[0:1, :]),
        "w0b": np.ascontiguousarray(W0[1:, :]),
        "w1": f(inputs["W1"]), "w2": f(inputs["W2"]), "wo": f(inputs["Wout"]),
        "lbT": np.ascontiguousarray(f(inputs["lambda_b"]).T),
        "cst": cst,
    }
    in_maps = []
    for c in range(NCORES):
        s = slice(c * BT, (c + 1) * BT)
        m = dict(shared)
        m["tn"] = np.ascontiguousarray(tnT[:, s])
        m["pT"] = np.ascontiguousarray(pT[:, s])
        in_maps.append(m)
    return in_maps


_NC_CACHE = {}


def _get_nc(mm_dt=MM_DT):
    key = str(mm_dt)
    if key not in _NC_CACHE:
        _NC_CACHE[key] = build_nc(mm_dt)
    return _NC_CACHE[key]


def run(inputs, trace=False, mm_dt=MM_DT):
    nc = _get_nc(mm_dt)
    in_maps = _host_prep(inputs)
    res = run_bass_kernel_spmd(nc, in_maps, list(range(NCORES)), trace=trace)
    out = np.concatenate([res.results[c]["outT"] for c in range(NCORES)], axis=1).T
    out_t = np.concatenate([res.results[c]["out_tT"] for c in range(NCORES)], axis=1).T
    phys = np.concatenate([res.results[c]["physT"] for c in range(NCORES)], axis=1).T
    outs = (np.ascontiguousarray(out), np.ascontiguousarray(out_t),
            np.ascontiguousarray(phys))
    return outs, res


def kernel(**inputs):
    outs, _ = run(inputs, trace=False)
    return outs


# revision 8
# speedup vs baseline: 1.2357x; 1.2357x over previous
"""Trainium2 Bass kernel for nn_PinnLayer (PINN power-grid layer).

Math (per batch row b, closed-form nested forward-mode AD wrt t):
  x = [tn, pn] in R^513, tn = 0.2*t - 1, pn = alpha*p + beta
  z1 = x W0 + b0;  zdot1 = 0.2*W0[0,:] =: r1 (const);  zddot1 = 0
  a  = tanh(z);  u = 1-a^2;  adot = u*zdot;  addot = u*zddot - 2*a*u*zdot^2
  (3 tanh layers), out = a3 Wout + bout, out_t = ad3 Wout, out_tt = add3 Wout
  conn_i = sum_j lb[i,j] sin(o_i - o_j) = sin(o_i)*(lb cos(o))_i - cos(o_i)*(lb sin(o))_i
  physics = lam_m*out_tt + lam_d*out_t + conn - p

Device layout: everything transposed — hidden/bus dim on partitions (4 chunks
of 128), batch on the free dim (128 per core, data-parallel over 8 cores).
Weights W[k_in, m_out] are used directly as matmul lhsT; activations never
need transposing. Per layer the rhs is the stacked [a | adot | addot]
(free=384) so each weight chunk is loaded once for all three matmuls.

The layer-1 matmul runs in exact fp32 (its inputs are the raw normalized
network inputs). Layers 2/3/out/conn optionally run in float32r (single-pass
PE streaming, ~4x the fp32 matmul rate at free>=256); all producers of those
matmul operands write float32r so the HW rounds consistently.
"""

import numpy as np

import concourse.bass as bass
import concourse.tile as tile
import concourse.mybir as mybir
from concourse import bacc
from concourse.bass import ts
from concourse.bass_utils import run_bass_kernel_spmd

F32 = mybir.dt.float32
F32R = mybir.dt.float32r
AF = mybir.ActivationFunctionType
OP = mybir.AluOpType

B, N, H = 1024, 512, 512
NCORES = 8
BT = B // NCORES          # 128 batch per core
C = 4                     # 128-partition chunks over H / N

# cst column layout: [128, 4]-shaped blocks at 4*i, then single columns
CB0, CB1, CB2, CBO, CLM, CLD, CAL, CBE, CR1, CS1 = (4 * i for i in range(10))
CZERO, CHALFPI, CONE = 40, 41, 42
NCST = 43

MM_DT = F32R


def build_nc(mm_dt=MM_DT):
    nc = bacc.Bacc("TRN2", target_bir_lowering=False, debug=False)

    tn_d = nc.dram_tensor("tn", [1, BT], F32, kind="ExternalInput").ap()
    pT_d = nc.dram_tensor("pT", [N, BT], F32, kind="ExternalInput").ap()
    w0r_d = nc.dram_tensor("w0r", [1, H], F32, kind="ExternalInput").ap()
    w0b_d = nc.dram_tensor("w0b", [N, H], F32, kind="ExternalInput").ap()
    w1_d = nc.dram_tensor("w1", [H, H], mm_dt, kind="ExternalInput").ap()
    w2_d = nc.dram_tensor("w2", [H, H], mm_dt, kind="ExternalInput").ap()
    wo_d = nc.dram_tensor("wo", [H, N], mm_dt, kind="ExternalInput").ap()
    lbT_d = nc.dram_tensor("lbT", [N, N], mm_dt, kind="ExternalInput").ap()
    cst_d = nc.dram_tensor("cst", [128, NCST], F32, kind="ExternalInput").ap()

    outT_d = nc.dram_tensor("outT", [N, BT], F32, kind="ExternalOutput").ap()
    out_tT_d = nc.dram_tensor("out_tT", [N, BT], F32, kind="ExternalOutput").ap()
    physT_d = nc.dram_tensor("physT", [N, BT], F32, kind="ExternalOutput").ap()

    # read-side view of an R-layer tile for non-matmul consumers: the bits
    # are already rounded, read them as plain fp32
    rd = (lambda ap: ap.bitcast(F32)) if mm_dt != F32 else (lambda ap: ap)

    with tile.TileContext(nc) as tc:
        with (
            tc.tile_pool(name="weights", bufs=1) as wp,
            tc.tile_pool(name="data", bufs=1) as dp,
            tc.tile_pool(name="scratch", bufs=4) as sp,
            tc.tile_pool(name="psum", bufs=8, space="PSUM") as pp,
        ):
            # ---- input DMAs (one per tensor; weights land as [128, C*out]) --
            cst = dp.tile([128, NCST], F32)
            nc.sync.dma_start(cst[:], cst_d)
            tn = dp.tile([1, BT], F32)
            nc.sync.dma_start(tn[:], tn_d)
            pT = dp.tile([128, C * BT], F32)
            nc.sync.dma_start(
                pT[:].rearrange("p (c b) -> p c b", b=BT),
                pT_d.rearrange("(c p) b -> p c b", p=128),
            )
            w0r = wp.tile([1, H], F32)
            nc.sync.dma_start(w0r[:], w0r_d)

            def load_w(name, d, dt):
                t = wp.tile([128, C * H], dt, tag=name)
                nc.sync.dma_start(
                    t[:].rearrange("p (c n) -> p c n", n=H),
                    d.rearrange("(c p) n -> p c n", p=128),
                )
                return t

            w0 = load_w("w0", w0b_d, F32)
            w1 = load_w("w1", w1_d, mm_dt)
            w2 = load_w("w2", w2_d, mm_dt)
            wo = load_w("wo", wo_d, mm_dt)
            lb = load_w("lb", lbT_d, mm_dt)

            def col(base, m=0):
                return cst[:, base + m : base + m + 1]

            zero = col(CZERO)
            halfpi = col(CHALFPI)
            one = col(CONE)

            # ---- normalize power: pn = alpha*p + beta ----------------------
            pn = dp.tile([128, C * BT], F32)
            for k in range(C):
                nc.vector.tensor_scalar(
                    pn[:, ts(k, BT)], pT[:, ts(k, BT)],
                    col(CAL, k), col(CBE, k), OP.mult, OP.add,
                )

            # ---- layer 1 (exact fp32 matmul) -------------------------------
            # R tiles hold [a | adot | addot] per chunk, free-stacked (384)
            R1 = dp.tile([128, C * 384], mm_dt, tag="R1")
            for m in range(C):
                ps = pp.tile([128, 384], F32, tag="ps")
                z = ps[:, 0:128]
                for k in range(C):
                    nc.tensor.matmul(
                        z, w0[:, k * H + m * 128 : k * H + (m + 1) * 128],
                        pn[:, ts(k, BT)], start=(k == 0), stop=False,
                    )
                nc.tensor.matmul(
                    z, w0r[0:1, ts(m, 128)], tn[0:1, :], start=False, stop=True,
                )
                A = R1[:, m * 384 : m * 384 + 128]
                D = R1[:, m * 384 + 128 : m * 384 + 256]
                DD = R1[:, m * 384 + 256 : m * 384 + 384]
                nc.scalar.activation(A, z, AF.Tanh, bias=col(CB0, m))
                sq = sp.tile([128, 128], F32, tag="sq")
                nc.scalar.activation(sq[:], rd(A), AF.Square, bias=zero)
                u = sp.tile([128, 128], F32, tag="u")
                nc.scalar.activation(u[:], sq[:], AF.Identity, bias=one, scale=-1.0)
                # adot = u * r1 ; addot = (adot * s1) * a  with s1 = -2*r1
                nc.vector.tensor_scalar_mul(D, u[:], col(CR1, m))
                d1 = sp.tile([128, 128], F32, tag="d1")
                nc.vector.tensor_scalar_mul(d1[:], rd(D), col(CS1, m))
                nc.gpsimd.tensor_mul(DD, d1[:], rd(A))

            # ---- layers 2, 3 (k-outer matmuls keep the PE warm) ------------
            Rp = R1
            for w, cb, rtag in ((w1, CB1, "R2"), (w2, CB2, "R3")):
                Rn = dp.tile([128, C * 384], mm_dt, tag=rtag)
                pss = [pp.tile([128, 384], F32, tag="ps", name=f"ps{rtag}{i}") for i in range(C)]
                for k in range(C):
                    for m in range(C):
                        nc.tensor.matmul(
                            pss[m][:],
                            w[:, k * H + m * 128 : k * H + (m + 1) * 128],
                            Rp[:, k * 384 : (k + 1) * 384],
                            start=(k == 0), stop=(k == C - 1),
                        )
                for m in range(C):
                    ps = pss[m]
                    z, zd, zdd = ps[:, 0:128], ps[:, 128:256], ps[:, 256:384]
                    A = Rn[:, m * 384 : m * 384 + 128]
                    D = Rn[:, m * 384 + 128 : m * 384 + 256]
                    DD = Rn[:, m * 384 + 256 : m * 384 + 384]
                    nc.scalar.activation(A, z, AF.Tanh, bias=col(cb, m))
                    sq = sp.tile([128, 128], F32, tag="sq")
                    nc.scalar.activation(sq[:], rd(A), AF.Square, bias=zero)
                    u = sp.tile([128, 128], F32, tag="u")
                    nc.scalar.activation(u[:], sq[:], AF.Identity, bias=one, scale=-1.0)
                    wz = sp.tile([128, 128], F32, tag="wz")
                    nc.vector.tensor_mul(wz[:], rd(A), zd)        # a*zd
                    nc.vector.tensor_mul(D, u[:], zd)             # adot = u*zd
                    t = sp.tile([128, 128], F32, tag="t")
                    nc.vector.tensor_mul(t[:], wz[:], rd(D))      # a*u*zd^2
                    dm = sp.tile([128, 128], F32, tag="dm")
                    nc.vector.tensor_mul(dm[:], u[:], zdd)        # u*zdd
                    nc.vector.scalar_tensor_tensor(
                        DD, t[:], -2.0, dm[:], OP.mult, OP.add)   # addot
                Rp = Rn

            # ---- output layer ----------------------------------------------
            O = dp.tile([128, C * BT], F32, tag="O")
            OT = dp.tile([128, C * BT], F32, tag="OT")
            PH = dp.tile([128, C * BT], F32, tag="PH")
            SC = dp.tile([128, C * 256], mm_dt, tag="SC")
            pss = [pp.tile([128, 384], F32, tag="ps", name=f"psout{i}") for i in range(C)]
            for k in range(C):
                for m in range(C):
                    nc.tensor.matmul(
                        pss[m][:],
                        wo[:, k * H + m * 128 : k * H + (m + 1) * 128],
                        Rp[:, k * 384 : (k + 1) * 384],
                        start=(k == 0), stop=(k == C - 1),
                    )
            for m in range(C):
                ps = pss[m]
                o = O[:, ts(m, BT)]
                ot = OT[:, ts(m, BT)]
                ph = PH[:, ts(m, BT)]
                nc.vector.tensor_scalar_add(o, ps[:, 0:128], col(CBO, m))
                nc.scalar.copy(ot, ps[:, 128:256])
                # ph = lam_m*out_tt - p, then += lam_d*out_t
                nc.vector.scalar_tensor_tensor(
                    ph, ps[:, 256:384], col(CLM, m), pT[:, ts(m, BT)],
                    OP.mult, OP.subtract)
                nc.vector.scalar_tensor_tensor(
                    ph, ps[:, 128:256], col(CLD, m), ph, OP.mult, OP.add)
                S = SC[:, m * 256 : m * 256 + 128]
                Cc = SC[:, m * 256 + 128 : m * 256 + 256]
                nc.scalar.activation(S, o, AF.Sin, bias=zero)
                nc.scalar.activation(Cc, o, AF.Sin, bias=halfpi)

            # ---- connectivity: conn = S*(lb C) - C*(lb S) -------------------
            pss2 = [pp.tile([128, 384], F32, tag="ps", name=f"psconn{i}") for i in range(C)]
            for k in range(C):
                for m in range(C):
                    nc.tensor.matmul(
                        pss2[m][:, 0:256],
                        lb[:, k * H + m * 128 : k * H + (m + 1) * 128],
                        SC[:, k * 256 : (k + 1) * 256],
                        start=(k == 0), stop=(k == C - 1),
                    )
            for m in range(C):
                ps2 = pss2[m]
                SMt, CMt = ps2[:, 0:128], ps2[:, 128:256]
                S = rd(SC[:, m * 256 : m * 256 + 128])
                Cc = rd(SC[:, m * 256 + 128 : m * 256 + 256])
                ph = PH[:, ts(m, BT)]
                q2 = sp.tile([128, 128], F32, tag="q2")
                nc.vector.tensor_mul(q2[:], S, CMt)
                nc.gpsimd.tensor_add(ph, ph, q2[:])
                q3 = sp.tile([128, 128], F32, tag="q3")
                nc.vector.tensor_mul(q3[:], Cc, SMt)
                nc.gpsimd.tensor_sub(ph, ph, q3[:])

            # ---- output DMAs -----------------------------------------------
            for d, t in ((outT_d, O), (out_tT_d, OT), (physT_d, PH)):
                nc.sync.dma_start(
                    d.rearrange("(c p) b -> p c b", p=128),
                    t[:].rearrange("p (c b) -> p c b", b=BT),
                )

    nc.compile()
    return nc


def _host_prep(inputs):
    f = lambda x: np.ascontiguousarray(np.asarray(x, dtype=np.float32))
    t = f(inputs["time_input"])          # [B,1]
    p = f(inputs["power_input"])         # [B,N]
    W0 = f(inputs["W0"])
    pl = f(inputs["p_lower"]).reshape(-1)
    pu = f(inputs["p_upper"]).reshape(-1)

    no_var = pu == pl
    denom = np.where(no_var, 1.0, pu - pl).astype(np.float32)
    alpha = np.where(no_var, 0.0, 2.0 / denom).astype(np.float32)
    beta = np.where(no_var, 0.0, -2.0 * pl / denom - 1.0).astype(np.float32)

    r1 = (0.2 * W0[0, :]).astype(np.float32)
    s1 = (-2.0 * r1).astype(np.float32)

    def colpack(v):
        return np.asarray(v, np.float32).reshape(C, 128).T  # [128, 4]

    blocks = [
        colpack(inputs["b0"]), colpack(inputs["b1"]), colpack(inputs["b2"]),
        colpack(inputs["bout"]), colpack(np.asarray(inputs["lambda_m"]).reshape(-1)),
        colpack(np.asarray(inputs["lambda_d"]).reshape(-1)),
        colpack(alpha), colpack(beta), colpack(r1), colpack(s1),
        np.zeros((128, 1), np.float32),
        np.full((128, 1), np.pi / 2, np.float32),
        np.ones((128, 1), np.float32),
    ]
    cst = np.ascontiguousarray(np.concatenate(blocks, axis=1).astype(np.float32))

    tnT = np.ascontiguousarray((0.2 * t - 1.0).T)        # [1, B]
    pT = np.ascontiguousarray(p.T)                        # [N, B]

    shared = {
        "w0r": np.ascontiguousarray(W0[0:1, :]),
        "w0b": np.ascontiguousarray(W0[1:, :]),
        "w1": f(inputs["W1"]), "w2": f(inputs["W2"]), "wo": f(inputs["Wout"]),
        "lbT": np.ascontiguousarray(f(inputs["lambda_b"]).T),
        "cst": cst,
    }
    in_maps = []
    for c in range(NCORES):
        s = slice(c * BT, (c + 1) * BT)
        m = dict(shared)
        m["tn"] = np.ascontiguousarray(tnT[:, s])
        m["pT"] = np.ascontiguousarray(pT[:, s])
        in_maps.append(m)
    return in_maps


_NC_CACHE = {}


def _get_nc(mm_dt=MM_DT):
    key = str(mm_dt)
    if key not in _NC_CACHE:
        _NC_CACHE[key] = build_nc(mm_dt)
    return _NC_CACHE[key]


def run(inputs, trace=False, mm_dt=MM_DT):
    nc = _get_nc(mm_dt)
    in_maps = _host_prep(inputs)
    res = run_bass_kernel_spmd(nc, in_maps, list(range(NCORES)), trace=trace)
    out = np.concatenate([res.results[c]["outT"] for c in range(NCORES)], axis=1).T
    out_t = np.concatenate([res.results[c]["out_tT"] for c in range(NCORES)], axis=1).T
    phys = np.concatenate([res.results[c]["physT"] for c in range(NCORES)], axis=1).T
    outs = (np.ascontiguousarray(out), np.ascontiguousarray(out_t),
            np.ascontiguousarray(phys))
    return outs, res


def kernel(**inputs):
    outs, _ = run(inputs, trace=False)
    return outs


# revision 13
# speedup vs baseline: 1.6292x; 1.3185x over previous
"""Trainium2 Bass kernel for nn_PinnLayer (PINN power-grid layer).

Math (per batch row b, closed-form nested forward-mode AD wrt t):
  x = [tn, pn] in R^513, tn = 0.2*t - 1, pn = alpha*p + beta
  z1 = x W0 + b0;  zdot1 = 0.2*W0[0,:] =: r1 (const);  zddot1 = 0
  a  = tanh(z);  u = 1-a^2;  adot = u*zdot;  addot = u*zddot - 2*a*u*zdot^2
  (3 tanh layers), out = a3 Wout + bout, out_t = ad3 Wout, out_tt = add3 Wout
  conn_i = sum_j lb[i,j] sin(o_i - o_j) = sin(o_i)*(lb cos(o))_i - cos(o_i)*(lb sin(o))_i
  physics = lam_m*out_tt + lam_d*out_t + conn - p

Device layout: everything transposed — hidden/bus dim on partitions (4 chunks
of 128), batch on the free dim (128 per core, data-parallel over 8 cores).
Weights W[k_in, m_out] are used directly as matmul lhsT; activations never
need transposing. Per layer the rhs is the stacked [a | adot | addot]
(free=384) so each weight chunk is loaded once for all three matmuls.

The layer-1 matmul runs in exact fp32 (its inputs are the raw normalized
network inputs). Layers 2/3/out/conn optionally run in float32r (single-pass
PE streaming, ~4x the fp32 matmul rate at free>=256); all producers of those
matmul operands write float32r so the HW rounds consistently.
"""

import numpy as np

import concourse.bass as bass
import concourse.tile as tile
import concourse.mybir as mybir
from concourse import bacc
from concourse.bass import ts
from concourse.bass_utils import run_bass_kernel_spmd

F32 = mybir.dt.float32
F32R = mybir.dt.float32r
AF = mybir.ActivationFunctionType
OP = mybir.AluOpType

B, N, H = 1024, 512, 512
NCORES = 8
BT = B // NCORES          # 128 batch per core
C = 4                     # 128-partition chunks over H / N

# cst column layout: [128, 4]-shaped blocks at 4*i, then single columns
CB0, CB1, CB2, CBO, CLM, CLD, CAL, CBE, CR1, CS1 = (4 * i for i in range(10))
CZERO, CHALFPI, CONE = 40, 41, 42
NCST = 43

MM_DT = F32R


def build_nc(mm_dt=MM_DT):
    nc = bacc.Bacc("TRN2", target_bir_lowering=False, debug=False)

    tn_d = nc.dram_tensor("tn", [1, BT], F32, kind="ExternalInput").ap()
    pT_d = nc.dram_tensor("pT", [N, BT], F32, kind="ExternalInput").ap()
    w0r_d = nc.dram_tensor("w0r", [1, H], F32, kind="ExternalInput").ap()
    w0b_d = nc.dram_tensor("w0b", [N, H], F32, kind="ExternalInput").ap()
    w1_d = nc.dram_tensor("w1", [H, H], mm_dt, kind="ExternalInput").ap()
    w2_d = nc.dram_tensor("w2", [H, H], mm_dt, kind="ExternalInput").ap()
    wo_d = nc.dram_tensor("wo", [H, N], mm_dt, kind="ExternalInput").ap()
    lbT_d = nc.dram_tensor("lbT", [N, N], mm_dt, kind="ExternalInput").ap()
    cst_d = nc.dram_tensor("cst", [128, NCST], F32, kind="ExternalInput").ap()

    outT_d = nc.dram_tensor("outT", [N, BT], F32, kind="ExternalOutput").ap()
    out_tT_d = nc.dram_tensor("out_tT", [N, BT], F32, kind="ExternalOutput").ap()
    physT_d = nc.dram_tensor("physT", [N, BT], F32, kind="ExternalOutput").ap()

    # read-side view of an R-layer tile for non-matmul consumers: the bits
    # are already rounded, read them as plain fp32
    rd = (lambda ap: ap.bitcast(F32)) if mm_dt != F32 else (lambda ap: ap)

    with tile.TileContext(nc) as tc:
        with (
            tc.tile_pool(name="weights", bufs=1) as wp,
            tc.tile_pool(name="data", bufs=1) as dp,
            tc.tile_pool(name="scratch", bufs=4) as sp,
            tc.tile_pool(name="psum", bufs=8, space="PSUM") as pp,
        ):
            # ---- input DMAs (one per tensor; weights land as [128, C*out]) --
            cst = dp.tile([128, NCST], F32)
            nc.sync.dma_start(cst[:], cst_d)
            tn = dp.tile([1, BT], F32)
            nc.sync.dma_start(tn[:], tn_d)
            w0r = wp.tile([1, H], F32)
            nc.sync.dma_start(w0r[:], w0r_d)
            w0 = wp.tile([128, C * H], F32, tag="w0")
            pT = dp.tile([128, C * BT], F32)
            for k in range(C):
                nc.sync.dma_start(
                    w0[:, k * H : (k + 1) * H], w0b_d[k * 128 : (k + 1) * 128, :])
                nc.sync.dma_start(
                    pT[:, ts(k, BT)], pT_d[k * 128 : (k + 1) * 128, :])

            def load_w(name, d, dt):
                t = wp.tile([128, C * H], dt, tag=name)
                nc.sync.dma_start(
                    t[:].rearrange("p (c n) -> p c n", n=H),
                    d.rearrange("(c p) n -> p c n", p=128),
                )
                return t

            w1 = load_w("w1", w1_d, mm_dt)
            w2 = load_w("w2", w2_d, mm_dt)
            wo = load_w("wo", wo_d, mm_dt)
            lb = load_w("lb", lbT_d, mm_dt)

            def col(base, m=0):
                return cst[:, base + m : base + m + 1]

            zero = col(CZERO)
            halfpi = col(CHALFPI)
            one = col(CONE)

            ones = dp.tile([128, 128], F32)
            nc.vector.memset(ones[:], 1.0)

            # ---- normalize power: pn = alpha*p + beta ----------------------
            pn = dp.tile([128, C * BT], F32)
            for k in range(C):
                nc.vector.tensor_scalar(
                    pn[:, ts(k, BT)], pT[:, ts(k, BT)],
                    col(CAL, k), col(CBE, k), OP.mult, OP.add,
                )

            # ---- layer 1 (exact fp32 matmul) -------------------------------
            # R tiles hold [a | adot | addot] per chunk, free-stacked (384)
            R1 = dp.tile([128, C * 384], mm_dt, tag="R1")
            l1ps = [pp.tile([128, 384], F32, tag="ps", name=f"psl1{i}")
                    for i in range(C)]
            for m in range(C):
                nc.tensor.matmul(
                    l1ps[m][:, 0:128], w0[:, m * 128 : m * 128 + 128],
                    pn[:, ts(0, BT)], start=True, stop=False)
            for m in range(C):
                z = l1ps[m][:, 0:128]
                for k in range(1, C):
                    nc.tensor.matmul(
                        z, w0[:, k * H + m * 128 : k * H + (m + 1) * 128],
                        pn[:, ts(k, BT)], start=False, stop=False,
                    )
                nc.tensor.matmul(
                    z, w0r[0:1, ts(m, 128)], tn[0:1, :], start=False, stop=True,
                )
                A = R1[:, m * 384 : m * 384 + 128]
                D = R1[:, m * 384 + 128 : m * 384 + 256]
                DD = R1[:, m * 384 + 256 : m * 384 + 384]
                nc.scalar.activation(A, z, AF.Tanh, bias=col(CB0, m))
                sq = sp.tile([128, 128], F32, tag="sq")
                nc.scalar.activation(sq[:], rd(A), AF.Square, bias=zero)
                u = sp.tile([128, 128], F32, tag="u")
                nc.gpsimd.tensor_sub(u[:], ones[:], sq[:])
                # adot = u * r1 ; addot = (adot * s1) * a  with s1 = -2*r1
                nc.vector.tensor_scalar_mul(D, u[:], col(CR1, m))
                d1 = sp.tile([128, 128], F32, tag="d1")
                nc.vector.tensor_scalar_mul(d1[:], rd(D), col(CS1, m))
                nc.gpsimd.tensor_mul(DD, d1[:], rd(A))

            # ---- layers 2, 3 (k-outer matmuls keep the PE warm) ------------
            Rp = R1
            for w, cb, rtag in ((w1, CB1, "R2"), (w2, CB2, "R3")):
                Rn = dp.tile([128, C * 384], mm_dt, tag=rtag)
                pss = [pp.tile([128, 384], F32, tag="ps", name=f"ps{rtag}{i}") for i in range(C)]
                for m in range(C):
                    nc.tensor.matmul(
                        pss[m][:], w[:, m * 128 : m * 128 + 128],
                        Rp[:, 0:384], start=True, stop=False)
                for m in range(C):
                    ps = pss[m]
                    for k in range(1, C):
                        nc.tensor.matmul(
                            ps[:],
                            w[:, k * H + m * 128 : k * H + (m + 1) * 128],
                            Rp[:, k * 384 : (k + 1) * 384],
                            start=False, stop=(k == C - 1),
                        )
                    z, zd, zdd = ps[:, 0:128], ps[:, 128:256], ps[:, 256:384]
                    A = Rn[:, m * 384 : m * 384 + 128]
                    D = Rn[:, m * 384 + 128 : m * 384 + 256]
                    DD = Rn[:, m * 384 + 256 : m * 384 + 384]
                    nc.scalar.activation(A, z, AF.Tanh, bias=col(cb, m))
                    sq = sp.tile([128, 128], F32, tag="sq")
                    nc.scalar.activation(sq[:], rd(A), AF.Square, bias=zero)
                    u = sp.tile([128, 128], F32, tag="u")
                    nc.gpsimd.tensor_sub(u[:], ones[:], sq[:])
                    wz = sp.tile([128, 128], F32, tag="wz")
                    nc.vector.tensor_mul(wz[:], rd(A), zd)        # a*zd
                    nc.vector.tensor_mul(D, u[:], zd)             # adot = u*zd
                    t = sp.tile([128, 128], F32, tag="t")
                    nc.gpsimd.tensor_mul(t[:], wz[:], rd(D))      # a*u*zd^2
                    dm = sp.tile([128, 128], F32, tag="dm")
                    nc.vector.tensor_mul(dm[:], u[:], zdd)        # u*zdd
                    nc.vector.scalar_tensor_tensor(
                        DD, t[:], -2.0, dm[:], OP.mult, OP.add)   # addot
                Rp = Rn

            # ---- output layer ----------------------------------------------
            O = dp.tile([128, C * BT], F32, tag="O")
            OT = dp.tile([128, C * BT], F32, tag="OT")
            PH = dp.tile([128, C * BT], F32, tag="PH")
            SC = dp.tile([128, C * 256], mm_dt, tag="SC")
            pss = [pp.tile([128, 384], F32, tag="ps", name=f"psout{i}") for i in range(C)]
            for m in range(C):
                nc.tensor.matmul(
                    pss[m][:], wo[:, m * 128 : m * 128 + 128],
                    Rp[:, 0:384], start=True, stop=False)
            for m in range(C):
                ps = pss[m]
                for k in range(1, C):
                    nc.tensor.matmul(
                        ps[:],
                        wo[:, k * H + m * 128 : k * H + (m + 1) * 128],
                        Rp[:, k * 384 : (k + 1) * 384],
                        start=False, stop=(k == C - 1),
                    )
                o = O[:, ts(m, BT)]
                ot = OT[:, ts(m, BT)]
                ph = PH[:, ts(m, BT)]
                nc.vector.tensor_scalar_add(o, ps[:, 0:128], col(CBO, m))
                nc.scalar.copy(ot, ps[:, 128:256])
                # ph = lam_m*out_tt - p, then += lam_d*out_t
                nc.vector.scalar_tensor_tensor(
                    ph, ps[:, 256:384], col(CLM, m), pT[:, ts(m, BT)],
                    OP.mult, OP.subtract)
                nc.vector.scalar_tensor_tensor(
                    ph, ps[:, 128:256], col(CLD, m), ph, OP.mult, OP.add)
                S = SC[:, m * 256 : m * 256 + 128]
                Cc = SC[:, m * 256 + 128 : m * 256 + 256]
                nc.scalar.activation(S, o, AF.Sin, bias=zero)
                nc.scalar.activation(Cc, o, AF.Sin, bias=halfpi)

            # ---- connectivity: conn = S*(lb C) - C*(lb S) -------------------
            pss2 = [pp.tile([128, 384], F32, tag="ps", name=f"psconn{i}") for i in range(C)]
            for m in range(C):
                nc.tensor.matmul(
                    pss2[m][:, 0:256], lb[:, m * 128 : m * 128 + 128],
                    SC[:, 0:256], start=True, stop=False)
            for m in range(C):
                ps2 = pss2[m]
                for k in range(1, C):
                    nc.tensor.matmul(
                        ps2[:, 0:256],
                        lb[:, k * H + m * 128 : k * H + (m + 1) * 128],
                        SC[:, k * 256 : (k + 1) * 256],
                        start=False, stop=(k == C - 1),
                    )
                SMt, CMt = ps2[:, 0:128], ps2[:, 128:256]
                S = rd(SC[:, m * 256 : m * 256 + 128])
                Cc = rd(SC[:, m * 256 + 128 : m * 256 + 256])
                ph = PH[:, ts(m, BT)]
                q2 = sp.tile([128, 128], F32, tag="q2")
                nc.vector.tensor_mul(q2[:], S, CMt)
                nc.gpsimd.tensor_add(ph, ph, q2[:])
                q3 = sp.tile([128, 128], F32, tag="q3")
                nc.vector.tensor_mul(q3[:], Cc, SMt)
                nc.gpsimd.tensor_sub(ph, ph, q3[:])

            # ---- output DMAs -----------------------------------------------
            for d, t in ((outT_d, O), (out_tT_d, OT), (physT_d, PH)):
                nc.sync.dma_start(
                    d.rearrange("(c p) b -> p c b", p=128),
                    t[:].rearrange("p (c b) -> p c b", b=BT),
                )

    nc.compile()
    return nc


def _host_prep(inputs):
    f = lambda x: np.ascontiguousarray(np.asarray(x, dtype=np.float32))
    t = f(inputs["time_input"])          # [B,1]
    p = f(inputs["power_input"])         # [B,N]
    W0 = f(inputs["W0"])
    pl = f(inputs["p_lower"]).reshape(-1)
    pu = f(inputs["p_upper"]).reshape(-1)

    no_var = pu == pl
    denom = np.where(no_var, 1.0, pu - pl).astype(np.float32)
    alpha = np.where(no_var, 0.0, 2.0 / denom).astype(np.float32)
    beta = np.where(no_var, 0.0, -2.0 * pl / denom - 1.0).astype(np.float32)

    r1 = (0.2 * W0[0, :]).astype(np.float32)
    s1 = (-2.0 * r1).astype(np.float32)

    def colpack(v):
        return np.asarray(v, np.float32).reshape(C, 128).T  # [128, 4]

    blocks = [
        colpack(inputs["b0"]), colpack(inputs["b1"]), colpack(inputs["b2"]),
        colpack(inputs["bout"]), colpack(np.asarray(inputs["lambda_m"]).reshape(-1)),
        colpack(np.asarray(inputs["lambda_d"]).reshape(-1)),
        colpack(alpha), colpack(beta), colpack(r1), colpack(s1),
        np.zeros((128, 1), np.float32),
        np.full((128, 1), np.pi / 2, np.float32),
        np.ones((128, 1), np.float32),
    ]
    cst = np.ascontiguousarray(np.concatenate(blocks, axis=1).astype(np.float32))

    tnT = np.ascontiguousarray((0.2 * t - 1.0).T)        # [1, B]
    pT = np.ascontiguousarray(p.T)                        # [N, B]

    shared = {
        "w0r": np.ascontiguousarray(W0[0:1, :]),
        "w0b": np.ascontiguousarray(W0[1:, :]),
        "w1": f(inputs["W1"]), "w2": f(inputs["W2"]), "wo": f(inputs["Wout"]),
        "lbT": np.ascontiguousarray(f(inputs["lambda_b"]).T),
        "cst": cst,
    }
    in_maps = []
    for c in range(NCORES):
        s = slice(c * BT, (c + 1) * BT)
        m = dict(shared)
        m["tn"] = np.ascontiguousarray(tnT[:, s])
        m["pT"] = np.ascontiguousarray(pT[:, s])
        in_maps.append(m)
    return in_maps


_NC_CACHE = {}


def _get_nc(mm_dt=MM_DT):
    key = str(mm_dt)
    if key not in _NC_CACHE:
        _NC_CACHE[key] = build_nc(mm_dt)
    return _NC_CACHE[key]


def run(inputs, trace=False, mm_dt=MM_DT):
    nc = _get_nc(mm_dt)
    in_maps = _host_prep(inputs)
    res = run_bass_kernel_spmd(nc, in_maps, list(range(NCORES)), trace=trace)
    out = np.concatenate([res.results[c]["outT"] for c in range(NCORES)], axis=1).T
    out_t = np.concatenate([res.results[c]["out_tT"] for c in range(NCORES)], axis=1).T
    phys = np.concatenate([res.results[c]["physT"] for c in range(NCORES)], axis=1).T
    outs = (np.ascontiguousarray(out), np.ascontiguousarray(out_t),
            np.ascontiguousarray(phys))
    return outs, res


def kernel(**inputs):
    outs, _ = run(inputs, trace=False)
    return outs


# revision 15
# speedup vs baseline: 1.7367x; 1.0660x over previous
"""Trainium2 Bass kernel for nn_PinnLayer (PINN power-grid layer).

Math (per batch row b, closed-form nested forward-mode AD wrt t):
  x = [tn, pn] in R^513, tn = 0.2*t - 1, pn = alpha*p + beta
  z1 = x W0 + b0;  zdot1 = 0.2*W0[0,:] =: r1 (const);  zddot1 = 0
  a  = tanh(z);  u = 1-a^2;  adot = u*zdot;  addot = u*zddot - 2*a*u*zdot^2
  (3 tanh layers), out = a3 Wout + bout, out_t = ad3 Wout, out_tt = add3 Wout
  conn_i = sum_j lb[i,j] sin(o_i - o_j) = sin(o_i)*(lb cos(o))_i - cos(o_i)*(lb sin(o))_i
  physics = lam_m*out_tt + lam_d*out_t + conn - p

Device layout: everything transposed — hidden/bus dim on partitions (4 chunks
of 128), batch on the free dim (128 per core, data-parallel over 8 cores).
Weights W[k_in, m_out] are used directly as matmul lhsT; activations never
need transposing. Per layer the rhs is the stacked [a | adot | addot]
(free=384) so each weight chunk is loaded once for all three matmuls.

The layer-1 matmul runs in exact fp32 (its inputs are the raw normalized
network inputs). Layers 2/3/out/conn optionally run in float32r (single-pass
PE streaming, ~4x the fp32 matmul rate at free>=256); all producers of those
matmul operands write float32r so the HW rounds consistently.
"""

import numpy as np

import concourse.bass as bass
import concourse.tile as tile
import concourse.mybir as mybir
from concourse import bacc
from concourse.bass import ts
from concourse.bass_utils import run_bass_kernel_spmd

F32 = mybir.dt.float32
F32R = mybir.dt.float32r
AF = mybir.ActivationFunctionType
OP = mybir.AluOpType

B, N, H = 1024, 512, 512
NCORES = 8
BT = B // NCORES          # 128 batch per core
C = 4                     # 128-partition chunks over H / N

# cst column layout: [128, 4]-shaped blocks at 4*i, then single columns
CB0, CB1, CB2, CBO, CLM, CLD, CAL, CBE, CR1, CS1 = (4 * i for i in range(10))
CZERO, CHALFPI, CONE = 40, 41, 42
NCST = 43

MM_DT = F32R


def build_nc(mm_dt=MM_DT):
    nc = bacc.Bacc("TRN2", target_bir_lowering=False, debug=False)

    tn_d = nc.dram_tensor("tn", [1, BT], F32, kind="ExternalInput").ap()
    pT_d = nc.dram_tensor("pT", [N, BT], F32, kind="ExternalInput").ap()
    w0r_d = nc.dram_tensor("w0r", [1, H], F32, kind="ExternalInput").ap()
    w0b_d = nc.dram_tensor("w0b", [N, H], F32, kind="ExternalInput").ap()
    w1_d = nc.dram_tensor("w1", [H, H], mm_dt, kind="ExternalInput").ap()
    w2_d = nc.dram_tensor("w2", [H, H], mm_dt, kind="ExternalInput").ap()
    wo_d = nc.dram_tensor("wo", [H, N], mm_dt, kind="ExternalInput").ap()
    lbT_d = nc.dram_tensor("lbT", [N, N], mm_dt, kind="ExternalInput").ap()
    cst_d = nc.dram_tensor("cst", [128, NCST], F32, kind="ExternalInput").ap()

    outT_d = nc.dram_tensor("outT", [N, BT], F32, kind="ExternalOutput").ap()
    out_tT_d = nc.dram_tensor("out_tT", [N, BT], F32, kind="ExternalOutput").ap()
    physT_d = nc.dram_tensor("physT", [N, BT], F32, kind="ExternalOutput").ap()

    # read-side view of an R-layer tile for non-matmul consumers: the bits
    # are already rounded, read them as plain fp32
    rd = (lambda ap: ap.bitcast(F32)) if mm_dt != F32 else (lambda ap: ap)

    with tile.TileContext(nc) as tc:
        with (
            tc.tile_pool(name="weights", bufs=1) as wp,
            tc.tile_pool(name="data", bufs=1) as dp,
            tc.tile_pool(name="scratch", bufs=4) as sp,
            tc.tile_pool(name="psum", bufs=8, space="PSUM") as pp,
        ):
            # ---- input DMAs (one per tensor; weights land as [128, C*out]) --
            cst = dp.tile([128, NCST], F32)
            nc.gpsimd.dma_start(cst[:], cst_d)
            tn = dp.tile([1, BT], F32)
            nc.gpsimd.dma_start(tn[:], tn_d)
            w0r = wp.tile([1, H], F32)
            nc.gpsimd.dma_start(w0r[:], w0r_d)
            w0 = wp.tile([128, C * H], F32, tag="w0")
            pT = dp.tile([128, C * BT], F32)
            nc.scalar.dma_start(pT[:, ts(0, BT)], pT_d[0:128, :])
            nc.sync.dma_start(w0[:, 0:H], w0b_d[0:128, :])
            for k in range(1, C):
                nc.sync.dma_start(
                    w0[:, k * H : (k + 1) * H], w0b_d[k * 128 : (k + 1) * 128, :])
                nc.sync.dma_start(
                    pT[:, ts(k, BT)], pT_d[k * 128 : (k + 1) * 128, :])

            def load_w(name, d, dt):
                t = wp.tile([128, C * H], dt, tag=name)
                nc.sync.dma_start(
                    t[:].rearrange("p (c n) -> p c n", n=H),
                    d.rearrange("(c p) n -> p c n", p=128),
                )
                return t

            w1 = load_w("w1", w1_d, mm_dt)
            w2 = load_w("w2", w2_d, mm_dt)
            wo = load_w("wo", wo_d, mm_dt)
            lb = load_w("lb", lbT_d, mm_dt)

            def col(base, m=0):
                return cst[:, base + m : base + m + 1]

            zero = col(CZERO)
            halfpi = col(CHALFPI)
            one = col(CONE)

            ones = dp.tile([128, 128], F32)
            nc.vector.memset(ones[:], 1.0)

            # ---- normalize power: pn = alpha*p + beta ----------------------
            pn = dp.tile([128, C * BT], F32)
            for k in range(C):
                nc.vector.tensor_scalar(
                    pn[:, ts(k, BT)], pT[:, ts(k, BT)],
                    col(CAL, k), col(CBE, k), OP.mult, OP.add,
                )

            # ---- layer 1 (exact fp32 matmul) -------------------------------
            # R tiles hold [a | adot | addot] per chunk, free-stacked (384)
            R1 = dp.tile([128, C * 384], mm_dt, tag="R1")
            l1ps = [pp.tile([128, 384], F32, tag="ps", name=f"psl1{i}")
                    for i in range(C)]
            for m in range(C):
                nc.tensor.matmul(
                    l1ps[m][:, 0:128], w0[:, m * 128 : m * 128 + 128],
                    pn[:, ts(0, BT)], start=True, stop=False)
            for m in range(C):
                z = l1ps[m][:, 0:128]
                for k in range(1, C):
                    nc.tensor.matmul(
                        z, w0[:, k * H + m * 128 : k * H + (m + 1) * 128],
                        pn[:, ts(k, BT)], start=False, stop=False,
                    )
                nc.tensor.matmul(
                    z, w0r[0:1, ts(m, 128)], tn[0:1, :], start=False, stop=True,
                )
                A = R1[:, m * 384 : m * 384 + 128]
                D = R1[:, m * 384 + 128 : m * 384 + 256]
                DD = R1[:, m * 384 + 256 : m * 384 + 384]
                nc.scalar.activation(A, z, AF.Tanh, bias=col(CB0, m))
                sq = sp.tile([128, 128], F32, tag="sq")
                nc.scalar.activation(sq[:], rd(A), AF.Square, bias=zero)
                u = sp.tile([128, 128], F32, tag="u")
                nc.gpsimd.tensor_sub(u[:], ones[:], sq[:])
                # adot = u * r1 ; addot = (adot * s1) * a  with s1 = -2*r1
                nc.vector.tensor_scalar_mul(D, u[:], col(CR1, m))
                d1 = sp.tile([128, 128], F32, tag="d1")
                nc.vector.tensor_scalar_mul(d1[:], rd(D), col(CS1, m))
                nc.gpsimd.tensor_mul(DD, d1[:], rd(A))

            # ---- layers 2, 3 (k-outer matmuls keep the PE warm) ------------
            Rp = R1
            for w, cb, rtag in ((w1, CB1, "R2"), (w2, CB2, "R3")):
                Rn = dp.tile([128, C * 384], mm_dt, tag=rtag)
                pss = [pp.tile([128, 384], F32, tag="ps", name=f"ps{rtag}{i}") for i in range(C)]
                for m in range(C):
                    nc.tensor.matmul(
                        pss[m][:], w[:, m * 128 : m * 128 + 128],
                        Rp[:, 0:384], start=True, stop=False)
                for m in range(C):
                    ps = pss[m]
                    for k in range(1, C):
                        nc.tensor.matmul(
                            ps[:],
                            w[:, k * H + m * 128 : k * H + (m + 1) * 128],
                            Rp[:, k * 384 : (k + 1) * 384],
                            start=False, stop=(k == C - 1),
                        )
                    z, zd, zdd = ps[:, 0:128], ps[:, 128:256], ps[:, 256:384]
                    A = Rn[:, m * 384 : m * 384 + 128]
                    D = Rn[:, m * 384 + 128 : m * 384 + 256]
                    DD = Rn[:, m * 384 + 256 : m * 384 + 384]
                    nc.scalar.activation(A, z, AF.Tanh, bias=col(cb, m))
                    sq = sp.tile([128, 128], F32, tag="sq")
                    nc.scalar.activation(sq[:], rd(A), AF.Square, bias=zero)
                    u = sp.tile([128, 128], F32, tag="u")
                    nc.gpsimd.tensor_sub(u[:], ones[:], sq[:])
                    wz = sp.tile([128, 128], F32, tag="wz")
                    nc.vector.tensor_mul(wz[:], rd(A), zd)        # a*zd
                    nc.vector.tensor_mul(D, u[:], zd)             # adot = u*zd
                    t = sp.tile([128, 128], F32, tag="t")
                    nc.gpsimd.tensor_mul(t[:], wz[:], rd(D))      # a*u*zd^2
                    dm = sp.tile([128, 128], F32, tag="dm")
                    nc.vector.tensor_mul(dm[:], u[:], zdd)        # u*zdd
                    nc.vector.scalar_tensor_tensor(
                        DD, t[:], -2.0, dm[:], OP.mult, OP.add)   # addot
                Rp = Rn

            # ---- output layer ----------------------------------------------
            O = dp.tile([128, C * BT], F32, tag="O")
            OT = dp.tile([128, C * BT], F32, tag="OT")
            PH = dp.tile([128, C * BT], F32, tag="PH")
            SC = dp.tile([128, C * 256], mm_dt, tag="SC")
            pss = [pp.tile([128, 384], F32, tag="ps", name=f"psout{i}") for i in range(C)]
            for m in range(C):
                nc.tensor.matmul(
                    pss[m][:], wo[:, m * 128 : m * 128 + 128],
                    Rp[:, 0:384], start=True, stop=False)
            for m in range(C):
                ps = pss[m]
                for k in range(1, C):
                    nc.tensor.matmul(
                        ps[:],
                        wo[:, k * H + m * 128 : k * H + (m + 1) * 128],
                        Rp[:, k * 384 : (k + 1) * 384],
                        start=False, stop=(k == C - 1),
                    )
                o = O[:, ts(m, BT)]
                ot = OT[:, ts(m, BT)]
                ph = PH[:, ts(m, BT)]
                nc.vector.tensor_scalar_add(o, ps[:, 0:128], col(CBO, m))
                nc.scalar.copy(ot, ps[:, 128:256])
                # ph = lam_m*out_tt - p, then += lam_d*out_t
                nc.vector.scalar_tensor_tensor(
                    ph, ps[:, 256:384], col(CLM, m), pT[:, ts(m, BT)],
                    OP.mult, OP.subtract)
                nc.vector.scalar_tensor_tensor(
                    ph, ps[:, 128:256], col(CLD, m), ph, OP.mult, OP.add)
                S = SC[:, m * 256 : m * 256 + 128]
                Cc = SC[:, m * 256 + 128 : m * 256 + 256]
                nc.scalar.activation(S, o, AF.Sin, bias=zero)
                nc.scalar.activation(Cc, o, AF.Sin, bias=halfpi)

            nc.sync.dma_start(
                outT_d.rearrange("(c p) b -> p c b", p=128),
                O[:].rearrange("p (c b) -> p c b", b=BT))
            nc.sync.dma_start(
                out_tT_d.rearrange("(c p) b -> p c b", p=128),
                OT[:].rearrange("p (c b) -> p c b", b=BT))

            # ---- connectivity: conn = S*(lb C) - C*(lb S) -------------------
            pss2 = [pp.tile([128, 384], F32, tag="ps", name=f"psconn{i}") for i in range(C)]
            for m in range(C):
                nc.tensor.matmul(
                    pss2[m][:, 0:256], lb[:, m * 128 : m * 128 + 128],
                    SC[:, 0:256], start=True, stop=False)
            for m in range(C):
                ps2 = pss2[m]
                for k in range(1, C):
                    nc.tensor.matmul(
                        ps2[:, 0:256],
                        lb[:, k * H + m * 128 : k * H + (m + 1) * 128],
                        SC[:, k * 256 : (k + 1) * 256],
                        start=False, stop=(k == C - 1),
                    )
                SMt, CMt = ps2[:, 0:128], ps2[:, 128:256]
                S = rd(SC[:, m * 256 : m * 256 + 128])
                Cc = rd(SC[:, m * 256 + 128 : m * 256 + 256])
                ph = PH[:, ts(m, BT)]
                q2 = sp.tile([128, 128], F32, tag="q2")
                nc.vector.tensor_mul(q2[:], S, CMt)
                nc.gpsimd.tensor_add(ph, ph, q2[:])
                q3 = sp.tile([128, 128], F32, tag="q3")
                nc.vector.tensor_mul(q3[:], Cc, SMt)
                nc.gpsimd.tensor_sub(ph, ph, q3[:])
                nc.sync.dma_start(physT_d[m * 128 : (m + 1) * 128, :], ph)

    nc.compile()
    return nc


def _host_prep(inputs):
    f = lambda x: np.ascontiguousarray(np.asarray(x, dtype=np.float32))
    t = f(inputs["time_input"])          # [B,1]
    p = f(inputs["power_input"])         # [B,N]
    W0 = f(inputs["W0"])
    pl = f(inputs["p_lower"]).reshape(-1)
    pu = f(inputs["p_upper"]).reshape(-1)

    no_var = pu == pl
    denom = np.where(no_var, 1.0, pu - pl).astype(np.float32)
    alpha = np.where(no_var, 0.0, 2.0 / denom).astype(np.float32)
    beta = np.where(no_var, 0.0, -2.0 * pl / denom - 1.0).astype(np.float32)

    r1 = (0.2 * W0[0, :]).astype(np.float32)
    s1 = (-2.0 * r1).astype(np.float32)

    def colpack(v):
        return np.asarray(v, np.float32).reshape(C, 128).T  # [128, 4]

    blocks = [
        colpack(inputs["b0"]), colpack(inputs["b1"]), colpack(inputs["b2"]),
        colpack(inputs["bout"]), colpack(np.asarray(inputs["lambda_m"]).reshape(-1)),
        colpack(np.asarray(inputs["lambda_d"]).reshape(-1)),
        colpack(alpha), colpack(beta), colpack(r1), colpack(s1),
        np.zeros((128, 1), np.float32),
        np.full((128, 1), np.pi / 2, np.float32),
        np.ones((128, 1), np.float32),
    ]
    cst = np.ascontiguousarray(np.concatenate(blocks, axis=1).astype(np.float32))

    tnT = np.ascontiguousarray((0.2 * t - 1.0).T)        # [1, B]
    pT = np.ascontiguousarray(p.T)                        # [N, B]

    shared = {
        "w0r": np.ascontiguousarray(W0[0:1, :]),
        "w0b": np.ascontiguousarray(W0[1:, :]),
        "w1": f(inputs["W1"]), "w2": f(inputs["W2"]), "wo": f(inputs["Wout"]),
        "lbT": np.ascontiguousarray(f(inputs["lambda_b"]).T),
        "cst": cst,
    }
    in_maps = []
    for c in range(NCORES):
        s = slice(c * BT, (c + 1) * BT)
        m = dict(shared)
        m["tn"] = np.ascontiguousarray(tnT[:, s])
        m["pT"] = np.ascontiguousarray(pT[:, s])
        in_maps.append(m)
    return in_maps


_NC_CACHE = {}


def _get_nc(mm_dt=MM_DT):
    key = str(mm_dt)
    if key not in _NC_CACHE:
        _NC_CACHE[key] = build_nc(mm_dt)
    return _NC_CACHE[key]


def run(inputs, trace=False, mm_dt=MM_DT):
    nc = _get_nc(mm_dt)
    in_maps = _host_prep(inputs)
    res = run_bass_kernel_spmd(nc, in_maps, list(range(NCORES)), trace=trace)
    out = np.concatenate([res.results[c]["outT"] for c in range(NCORES)], axis=1).T
    out_t = np.concatenate([res.results[c]["out_tT"] for c in range(NCORES)], axis=1).T
    phys = np.concatenate([res.results[c]["physT"] for c in range(NCORES)], axis=1).T
    outs = (np.ascontiguousarray(out), np.ascontiguousarray(out_t),
            np.ascontiguousarray(phys))
    return outs, res


def kernel(**inputs):
    outs, _ = run(inputs, trace=False)
    return outs
